# revision 1
# baseline (speedup 1.0000x reference)
"""Trainium2 Bass kernel for BasicCNN+LSTM (conv3x3+ReLU+GAP -> custom LSTM scan).

Self-contained: hardcodes shapes/sharding. Data-parallel over batch B=8 across
8 NeuronCores; each core processes one batch element end-to-end, the host
gathers the 8 [1,32] results.

Per-core device pipeline (per frame t of 24):
  - DMA a host-prepacked, channel-deinterleaved "stack" [36, 56*112] bf16 into
    an SBUF row-band (partition p = c*12 + dx*4 + r holds plane c shifted by
    (dx-1, parity row r)); 2 round-robin bands at partitions {0, 64} -> 2
    concurrent PE row-group streams.
  - Conv as ONE K=36 matmul per PSUM tile (contraction over the full 3x3x3
    receptive field of a vertically-packed pixel pair; M=96 = 2 px x 48
    filters, N=448 = 4 ja-blocks x 112 w, contiguous rhs). 14 tiles/frame.
  - Fused ReLU(+conv-bias)+GAP: ScalarE activation(Relu, bias, accum_out) and
    VectorE tensor_scalar((x+bias) max 0, accum_out), 7/7 split with separate
    per-engine gsum tiles (avoids cross-engine WAW serialization).
  - Tiny LSTM-ish scan step on-chip ([1,96] gates in free layout; the
    reference's state-order swap bug is reproduced faithfully). Scan step t
    is emitted after frame t+2's conv to avoid FIFO head-of-line blocking.
"""
import sys
if '/opt/trn_rl_repo' not in sys.path:
    sys.path.insert(0, '/opt/trn_rl_repo')

import numpy as np
import ml_dtypes

import concourse.bass as bass
import concourse.mybir as mybir
import concourse.tile as tile
from concourse.vector_clock import ScopedClock
from concourse.bass_utils import run_bass_kernel_spmd

# ---------------------------------------------------------------- constants
B, T, H, W, C, F, U = 8, 24, 112, 112, 3, 48, 32
JA = 56            # vertical pixel-pair blocks (112 rows / 2)
KP = 36            # stack partitions: 3 c x 3 dx x 4 window rows
M = 96             # 2 pixels x 48 filters
NSB = 14           # superblocks (PSUM tiles) per frame
NQ = 448           # columns per superblock = 4 ja-blocks x 112 w
FREE = JA * W      # stack free size per partition (elements)

FP32 = mybir.dt.float32
BF16 = mybir.dt.bfloat16

LAST_RESULTS = None  # BassKernelResults of the most recent run (for test.py)

# ------------------------------------------------- TileContext drain patch
# The container's walrus rejects >1 semaphore wait per instruction; Tile's
# kernel-tail drain aggregates all end-of-kernel waits onto one Drain.
# Spread them across single-wait NOPs on the sync engine instead.
def _patched_drain_and_barrier(self, tick_clock, wait_clock):
    nc = self.nc
    probe = nc.sync.nop(nofuse=True, hint="tail_waits")
    wait_clock.add_sem_waits(probe.ins, ScopedClock({None: tick_clock.global_clock}))
    waits = list(probe.ins.sync_info.on_wait or [])
    if len(waits) > 1:
        probe.ins.sync_info.on_wait = waits[:1]
        for i in range(1, len(waits)):
            extra = nc.sync.nop(nofuse=True, hint=f"tail_waits_{i}")
            si = extra.ins.sync_info
            if si is None:
                extra.ins.sync_info = mybir.SyncInfo(on_wait=[waits[i]], on_update=[])
            else:
                si.on_wait = [waits[i]]
    nc.sync.drain()
    nc.all_engine_barrier()
    popped = nc._tile_sem_poison_stack.pop()
    assert popped is self._sem_poison
    nc.clear_and_free_semaphores(list(self.sems.allocated().values()))
    nc.all_engine_barrier()


tile.TileContext._drain_and_barrier = _patched_drain_and_barrier

# Same walrus restriction for regular instructions: spill extra sem waits
# onto preceding same-engine NOPs at commit time.
_orig_commit = tile.TileContext._commit_instruction


def _patched_commit(self, inst, *args, **kwargs):
    si = getattr(inst, 'sync_info', None)
    if si is not None and si.on_wait and len(si.on_wait) > 1 \
            and inst.engine != mybir.EngineType.Unassigned:
        waits = list(si.on_wait)
        si.on_wait = waits[-1:]
        for w in waits[:-1]:
            nop = mybir.InstNoOp(
                name=self.nc.get_next_instruction_name(),
                ins=[], outs=[], bass_is_fusable=False)
            nop.engine = inst.engine
            nop.sync_info = mybir.SyncInfo(on_wait=[w], on_update=[])
            _orig_commit(self, nop, *args, **kwargs)
    return _orig_commit(self, inst, *args, **kwargs)


tile.TileContext._commit_instruction = _patched_commit

# NOTE: tried --enable-ldw-opt=true to dedupe the per-matmul stationary
# reloads (~70us of PE time); this walrus build fails in visitInstLdweights
# with it enabled, so the flag stays off.


# ------------------------------------------------------------- device code
def _build_bass(use_gbias=True):
    _build_bass.use_gbias = use_gbias
    nc = bass.Bass('TRN2', target_bir_lowering=False, debug=False)

    xin = nc.dram_tensor('xin', [T, KP, FREE], BF16, kind='ExternalInput')
    smat_d = nc.dram_tensor('smat', [KP, M], BF16, kind='ExternalInput')
    cbias_d = nc.dram_tensor('cbias', [M, 1], FP32, kind='ExternalInput')
    wfeat_d = nc.dram_tensor('wfeat', [M, 96], FP32, kind='ExternalInput')
    whid_d = nc.dram_tensor('whid', [U, 96], FP32, kind='ExternalInput')
    gbias_d = nc.dram_tensor('gbias', [1, 96], FP32, kind='ExternalInput')
    outh_d = nc.dram_tensor('outh', [1, U], FP32, kind='ExternalOutput')

    Relu = mybir.ActivationFunctionType.Relu
    Sigmoid = mybir.ActivationFunctionType.Sigmoid
    Tanh = mybir.ActivationFunctionType.Tanh
    Amax = mybir.AluOpType.max
    Aadd = mybir.AluOpType.add

    with tile.TileContext(nc) as tc:
        const = tc.alloc_tile_pool(name='const', bufs=1)
        state = tc.alloc_tile_pool(name='state', bufs=1)
        stackp = tc.alloc_tile_pool(name='stack', bufs=3)
        psum = tc.alloc_tile_pool(name='psum', bufs=3, space='PSUM')
        spsum = tc.alloc_tile_pool(name='spsum', bufs=2, space='PSUM')
        gs = tc.alloc_tile_pool(name='gs', bufs=6)
        fs = tc.alloc_tile_pool(name='fs', bufs=6)
        ga_pool = tc.alloc_tile_pool(name='ga', bufs=4)
        tmp = tc.alloc_tile_pool(name='tmp', bufs=6)

        # constants
        sc_all = const.tile([128, M], BF16, tag='sc')
        for s in range(2):
            nc.sync.dma_start(sc_all[64 * s:64 * s + KP, :], smat_d[:])
        cbias = const.tile([M, 1], FP32, tag='cb')
        nc.sync.dma_start(cbias[:], cbias_d[:])
        wfeat = const.tile([M, 96], FP32, tag='wf')
        nc.sync.dma_start(wfeat[:], wfeat_d[:])
        whid = const.tile([U, 96], FP32, tag='wh')
        nc.sync.dma_start(whid[:], whid_d[:])
        gbias = const.tile([1, 96], FP32, tag='gb')
        nc.sync.dma_start(gbias[:], gbias_d[:])
        zeros1k = const.tile([M, 1024], FP32, tag='z1k')
        nc.vector.memset(zeros1k[:], 0.0)

        # persistent scan state. new_cell lives in row 0 of a 32x32 block so
        # a single DVE 32x32 transpose yields its partition-form in cpblk's
        # column 0 (cheaper than PE transpose + PSUM copy-back).
        cellblk = state.tile([32, 32], FP32, tag='cellblk')
        cpblk = state.tile([32, 32], FP32, tag='cpblk')
        cellv = cellblk[0:1, 0:U]                       # prev new_cell
        cell_part = cpblk[0:U, 0:1]                     # new_cell, transposed
        hidv = state.tile([1, U], FP32, tag='hid')      # prev new_hidden
        nc.vector.memset(cellblk[:], 0.0)
        nc.vector.memset(cpblk[:], 0.0)
        nc.vector.memset(hidv[:], 0.0)

        fsums = [None] * T

        # Six 1024-col double-bank PSUM windows (2 matmuls, ONE fused
        # relu+accum each: halves the per-instruction + accumulator-read
        # overhead) + one 128-col tail, per frame. ACT 3 / DVE 3+tail.
        rounds = [None] * (T // 2)

        def get_round(g):
            if rounds[g] is None:
                rt = stackp.tile([128, FREE], BF16, tag='stk')
                nc.sync.dma_start(rt[0:KP, :], xin[2 * g])
                nc.sync.dma_start(rt[64:64 + KP, :], xin[2 * g + 1])
                rounds[g] = rt
            return rounds[g]

        def emit_conv(t):
            s = t % 2
            rt = get_round(t // 2)
            if s == 0 and t // 2 + 1 < T // 2:
                get_round(t // 2 + 1)  # prefetch next round's DMAs
            band = rt[64 * s:64 * s + KP, :]
            lhsT = sc_all[64 * s:64 * s + KP, :]

            gsumA = gs.tile([M, 4], FP32, tag='gsumA')
            gsumB = gs.tile([M, 4], FP32, tag='gsumB')
            zv = zeros1k.rearrange("p (b n) -> p b n", b=2)[:, :, 0:448]
            ia = ib = 0
            for k in range(7):
                # two 448-col matmuls at bank-aligned offsets 0/512, then one
                # fused relu+accum over the strided [96, 2, 448] view.
                # Engine assignment alternates with frame parity (3.5/3.5).
                ps = psum.tile([M, 1024], FP32, tag='ps')
                for h in range(2):
                    off = k * 896 + h * 448
                    nc.tensor.matmul(ps[:, h * 512:h * 512 + 448], lhsT,
                                     band[:, off:off + 448],
                                     start=True, stop=True,
                                     tile_position=(64 * s, 0))
                psv = ps.rearrange("p (b n) -> p b n", b=2)[:, :, 0:448]
                if (k + t) % 2 == 0:
                    nc.scalar.activation(psv, psv, Relu, bias=cbias[:],
                                         accum_out=gsumA[:, ia:ia + 1])
                    ia += 1
                else:
                    nc.vector.scalar_tensor_tensor(
                        out=psv, in0=psv, scalar=cbias[:],
                        in1=zv, op0=Aadd, op1=Amax,
                        accum_out=gsumB[:, ib:ib + 1])
                    ib += 1

            fsA = tmp.tile([M, 1], FP32, tag='fsA')
            nc.vector.reduce_sum(fsA[:], gsumA[:, 0:ia], axis=mybir.AxisListType.X)
            fsB = tmp.tile([M, 1], FP32, tag='fsB')
            nc.vector.reduce_sum(fsB[:], gsumB[:, 0:ib], axis=mybir.AxisListType.X)
            fsum = fs.tile([M, 1], FP32, tag='fsum')
            nc.vector.tensor_add(fsum[:], fsA[:], fsB[:])
            fsums[t] = fsum

        def emit_scan(t):
            # z-hidden part = prev new_cell (reference's state-order swap bug)
            fsum = fsums[t]
            pg = spsum.tile([1, 96], FP32, tag='sps')
            nc.tensor.matmul(pg[:], fsum[:], wfeat[:], start=True, stop=False)
            nc.tensor.matmul(pg[:], cell_part, whid[:], start=False, stop=True)
            if _build_bass.use_gbias:
                gpre = ga_pool.tile([1, 96], FP32, tag='gpre')
                nc.vector.tensor_add(gpre[:], pg[:], gbias[:])
            else:
                gpre = pg
            ga = ga_pool.tile([1, 96], FP32, tag='ga')
            nc.scalar.activation(ga[:, 0:2 * U], gpre[:, 0:2 * U], Sigmoid)
            nc.scalar.activation(ga[:, 2 * U:3 * U], gpre[:, 2 * U:3 * U], Tanh)
            t1 = tmp.tile([1, U], FP32, tag='t1')
            nc.vector.tensor_mul(t1[:], ga[:, 0:U], hidv[:])       # sig1*prev_hid
            t2 = tmp.tile([1, U], FP32, tag='t2')
            nc.vector.tensor_mul(t2[:], ga[:, U:2 * U], ga[:, 2 * U:3 * U])
            nc.vector.tensor_add(cellv, t1[:], t2[:])              # new_cell
            t3 = tmp.tile([1, U], FP32, tag='t3')
            nc.scalar.activation(t3[:], cellv, Tanh)
            nc.vector.tensor_mul(hidv[:], cellv, t3[:])            # new_hidden
            if t < T - 1:
                nc.vector.transpose(cpblk[:], cellblk[:])

        LAG = 4  # scan step t emitted alongside frame t+LAG's conv: its
        # deps are LAG frames old, so it never head-of-line blocks a queue.
        for t in range(T):
            if t >= LAG:
                emit_scan(t - LAG)
            emit_conv(t)
        for t in range(T - LAG, T):
            emit_scan(t)

        nc.sync.dma_start(outh_d[:], hidv[:])

        for p in (tmp, ga_pool, fs, gs, spsum, psum, stackp, state, const):
            p.release()

    return nc


# -------------------------------------------------------------- host prep
def _prep_inputs(x, conv_w, conv_b, W1, b1, W2, b2, W3, b3):
    x = np.asarray(x, np.float32)
    conv_w = np.asarray(conv_w, np.float32)
    conv_b = np.asarray(conv_b, np.float32)

    xp = np.zeros((B, T, H + 2, W + 2, C), np.float32)
    xp[:, :, 1:H + 1, 1:W + 1, :] = x
    xin2 = np.empty((B, T, KP, JA, W), np.float32)
    rows = 2 * np.arange(JA)
    for c in range(3):
        for dx in range(3):
            for r in range(4):
                p = c * 12 + dx * 4 + r
                xin2[:, :, p] = np.moveaxis(
                    xp[:, :, rows + r, dx:dx + W, c], 0, 2)
    xin2 = xin2.reshape(B, T, KP, FREE).astype(ml_dtypes.bfloat16)

    smat = np.zeros((KP, M), np.float32)
    for c in range(3):
        for dx in range(3):
            for r in range(4):
                p = c * 12 + dx * 4 + r
                for i in range(2):
                    dy = r - i
                    if 0 <= dy <= 2:
                        smat[p, i * F:(i + 1) * F] = conv_w[dy, dx, c, :]
    smat = smat.astype(ml_dtypes.bfloat16)
    cbias = np.concatenate([conv_b, conv_b]).reshape(M, 1).astype(np.float32)

    wfeat = np.zeros((M, 96), np.float32)
    whid = np.zeros((U, 96), np.float32)
    for g, Wg in enumerate([W1, W2, W3]):
        Wg = np.asarray(Wg, np.float32)
        for i in range(2):
            wfeat[i * F:(i + 1) * F, g * U:(g + 1) * U] = Wg[0:F, :] / float(H * W)
        whid[:, g * U:(g + 1) * U] = Wg[F:F + U, :]
    gbias = np.concatenate([np.asarray(b, np.float32) for b in (b1, b2, b3)])
    gbias = gbias.reshape(1, 96)

    return xin2, smat, cbias, wfeat, whid, gbias


# ------------------------------------------------------------------ kernel
def kernel(x, conv_w, conv_b, W1, b1, W2, b2, W3, b3, W4, b4):
    global LAST_RESULTS
    xin2, smat, cbias, wfeat, whid, gbias = _prep_inputs(
        x, conv_w, conv_b, W1, b1, W2, b2, W3, b3)

    nc = _build_bass(use_gbias=bool(np.any(gbias)))
    in_maps = [{
        'xin': np.ascontiguousarray(xin2[b]),
        'smat': smat,
        'cbias': cbias,
        'wfeat': wfeat,
        'whid': whid,
        'gbias': gbias,
    } for b in range(B)]

    res = run_bass_kernel_spmd(nc, in_maps, core_ids=list(range(B)))
    LAST_RESULTS = res
    out = np.stack([res.results[b]['outh'][0] for b in range(B)], axis=0)
    return out.astype(np.float32)



# revision 10
# speedup vs baseline: 1.0098x; 1.0098x over previous
"""Trainium2 Bass kernel for BasicCNN+LSTM (conv3x3+ReLU+GAP -> custom LSTM scan).

Self-contained: hardcodes shapes/sharding. Data-parallel over batch B=8 across
8 NeuronCores; each core processes one batch element end-to-end, the host
gathers the 8 [32] results.

v2 design (vs. baseline's 2-pixel bf16 scheme):
  - Conv as fp8(e4m3) DoubleRow matmuls: 8-pixel vertical blocks, K=90 taps
    (3c x 3dx x 10 window rows) split into 2 k-tiles of 45, M=128 = 16
    filters x 8 pixel positions (f-major), 3 filter groups. Moving columns
    per frame: 3 x 1568 (25% less than baseline, at 2x fp8 column rate),
    robust to the PE HAM clock gate (cold 1.2GHz still fits).
  - ReLU(+bias)+GAP as ONE instruction per (frame, filter-group) over a
    strided [128, 4, 392] view of a 4-bank PSUM tile, alternating
    Scalar ACT / Vector tensor_scalar, each with a bf16 accum_out column.
  - Scan step: 4 tiny accumulating matmuls (3x K=128 reading the gsum
    columns directly - the stationary replicates weights across the 8 pixel
    positions so the cross-partition fold is absorbed - plus K=32 for the
    cell part), one sigmoid ACT with per-partition scale [1,1,2] and bias
    [b1,b2,2*b3] (tanh(a)=2*sigmoid(2a)-1), and 6 tiny ALU ops spread over
    GpSimd/Vector. Reference's state-order swap bug kept: the z "hidden"
    input is the previous cell, and gate1 multiplies the previous hidden.
"""
import sys
if '/opt/trn_rl_repo' not in sys.path:
    sys.path.insert(0, '/opt/trn_rl_repo')

import numpy as np
import ml_dtypes

import concourse.bass as bass
import concourse.mybir as mybir
import concourse.tile as tile
from concourse.vector_clock import ScopedClock
from concourse.bass_utils import run_bass_kernel_spmd

# ---------------------------------------------------------------- constants
B, T, H, W, C, F, U = 8, 24, 112, 112, 3, 48, 32
KP = 45            # k' per k-tile (2 tiles of 45 = 90 taps: 3c x 3dx x 10wr)
JA = 14            # vertical 8-pixel blocks per frame (112/8)
NCOL = JA * W      # 1568 moving columns per frame per filter group
NCH = 392          # columns per matmul chunk (4 chunks, bank-aligned @512)
M = 128            # out partitions = 16 filters x 8 pixel positions
HWN = float(H * W)

FP32 = mybir.dt.float32
BF16 = mybir.dt.bfloat16
FP8 = mybir.dt.float8e4

LAST_RESULTS = None  # BassKernelResults of the most recent run (for test.py)

# ------------------------------------------------- TileContext drain patch
# The container's walrus rejects >1 semaphore wait per instruction; Tile's
# kernel-tail drain aggregates all end-of-kernel waits onto one Drain.
# Spread them across single-wait NOPs on the sync engine instead.
def _patched_drain_and_barrier(self, tick_clock, wait_clock):
    nc = self.nc
    probe = nc.sync.nop(nofuse=True, hint="tail_waits")
    wait_clock.add_sem_waits(probe.ins, ScopedClock({None: tick_clock.global_clock}))
    waits = list(probe.ins.sync_info.on_wait or [])
    if len(waits) > 1:
        probe.ins.sync_info.on_wait = waits[:1]
        for i in range(1, len(waits)):
            extra = nc.sync.nop(nofuse=True, hint=f"tail_waits_{i}")
            si = extra.ins.sync_info
            if si is None:
                extra.ins.sync_info = mybir.SyncInfo(on_wait=[waits[i]], on_update=[])
            else:
                si.on_wait = [waits[i]]
    nc.sync.drain()
    nc.all_engine_barrier()
    popped = nc._tile_sem_poison_stack.pop()
    assert popped is self._sem_poison
    nc.clear_and_free_semaphores(list(self.sems.allocated().values()))
    nc.all_engine_barrier()


tile.TileContext._drain_and_barrier = _patched_drain_and_barrier

# Same walrus restriction for regular instructions: spill extra sem waits
# onto preceding same-engine NOPs at commit time.
_orig_commit = tile.TileContext._commit_instruction


def _patched_commit(self, inst, *args, **kwargs):
    si = getattr(inst, 'sync_info', None)
    if si is not None and si.on_wait and len(si.on_wait) > 1 \
            and inst.engine != mybir.EngineType.Unassigned:
        waits = list(si.on_wait)
        si.on_wait = waits[-1:]
        for w in waits[:-1]:
            nop = mybir.InstNoOp(
                name=self.nc.get_next_instruction_name(),
                ins=[], outs=[], bass_is_fusable=False)
            nop.engine = inst.engine
            nop.sync_info = mybir.SyncInfo(on_wait=[w], on_update=[])
            _orig_commit(self, nop, *args, **kwargs)
    return _orig_commit(self, inst, *args, **kwargs)


tile.TileContext._commit_instruction = _patched_commit


# ------------------------------------------------------------- device code
def _build_bass():
    nc = bass.Bass('TRN2', target_bir_lowering=False, debug=False)

    xin = nc.dram_tensor('xin', [T, KP, 2 * NCOL], FP8, kind='ExternalInput')
    smat_d = nc.dram_tensor('smat', [KP, 2, 3 * M], FP8, kind='ExternalInput')
    cbias_d = nc.dram_tensor('cbias', [M, 3], FP32, kind='ExternalInput')
    wx_d = nc.dram_tensor('wx', [M, 3 * 96], BF16, kind='ExternalInput')
    wc_d = nc.dram_tensor('wc', [U + 1, 96], BF16, kind='ExternalInput')
    outh_d = nc.dram_tensor('outh', [U, 1], FP32, kind='ExternalOutput')

    Relu = mybir.ActivationFunctionType.Relu
    Sigmoid = mybir.ActivationFunctionType.Sigmoid
    Copy = mybir.ActivationFunctionType.Copy
    Amax = mybir.AluOpType.max
    Aadd = mybir.AluOpType.add
    Asub = mybir.AluOpType.subtract
    Amul = mybir.AluOpType.mult
    DR = mybir.MatmulPerfMode.DoubleRow

    with tile.TileContext(nc) as tc:
        const = tc.alloc_tile_pool(name='const', bufs=1)
        state = tc.alloc_tile_pool(name='state', bufs=3)
        stackp = tc.alloc_tile_pool(name='stack', bufs=4)
        psum = tc.alloc_tile_pool(name='psum', bufs=2, space='PSUM')
        gs = tc.alloc_tile_pool(name='gs', bufs=5)
        ga_pool = tc.alloc_tile_pool(name='ga', bufs=3)
        tmp = tc.alloc_tile_pool(name='tmp', bufs=4)

        # constants
        smat = const.tile([KP, 2, 3 * M], FP8, tag='sm')
        nc.sync.dma_start(smat[:], smat_d[:])
        cbias = const.tile([M, 3], FP32, tag='cb')
        nc.sync.dma_start(cbias[:], cbias_d[:])
        wx = const.tile([M, 3, 96], BF16, tag='wx')
        nc.sync.dma_start(wx[:], wx_d[:].rearrange("p (g n) -> p g n", g=3))
        wc = const.tile([U + 1, 96], BF16, tag='wc')
        nc.sync.dma_start(wc[:], wc_d[:])
        zer = const.tile([M, NCOL], FP32, tag='zer')
        nc.vector.memset(zer[:], 0.0)
        zv = zer[:].rearrange("p (u q) -> p u q", u=4)

        # scan state: cell [33,1] (row 32 = const 1.0 feeds the bias row of
        # wc), hidden [32,1]; manual 3-deep rotation of persistent tiles.
        cbufs = [state.tile([U + 1, 1], BF16, tag=f'c{i}', name=f'cbuf{i}')
                 for i in range(3)]
        hbufs = [state.tile([U, 1], BF16, tag=f'h{i}', name=f'hbuf{i}')
                 for i in range(3)]
        for cb in cbufs:
            nc.vector.memset(cb[:], 0.0)
            nc.vector.memset(cb[U:U + 1, :], 1.0)
        for hb in hbufs:
            nc.vector.memset(hb[:], 0.0)
        cprev, hprev = cbufs[2], hbufs[2]

        stacks = [None] * T
        gsums = [None] * T

        def get_stack(t):
            if stacks[t] is None:
                st = stackp.tile([KP, 2 * NCOL], FP8, tag='stk')
                nc.sync.dma_start(st[:], xin[t])
                stacks[t] = st
            return stacks[t]

        # Greedy static S/V balance for the 72 ReLU+GAP instructions.
        # Per-instruction serial cost model (ns): ACT (172+1568)/1.2 + acc
        # 187 + dec 32 = 1669; DVE TS (120+1568)/0.96 + acc 136 + dec 45 =
        # 1939. Scalar additionally runs 48 scan sigmoids (~10.2us), Vector
        # ~5.8us of scan ALU.
        assign = []
        s_acc, v_acc = 10200.0, 5800.0
        for _ in range(3 * T):
            if s_acc + 1669 <= v_acc + 1939:
                assign.append('S')
                s_acc += 1669
            else:
                assign.append('V')
                v_acc += 1939

        def emit_conv(t):
            st = get_stack(t)
            if t + 2 < T:
                get_stack(t + 2)  # prefetch
            stv = st.rearrange("p (h q) -> p h q", h=2)
            gsum = gs.tile([M, 3], BF16, tag='gsum')
            gsums[t] = gsum
            pt_g0 = None
            for g in range(3):
                pt = psum.tile([M, 2048], FP32, tag='pt')
                if g == 0:
                    pt_g0 = pt
                for k in range(4):
                    nc.tensor.matmul(
                        pt[:, 512 * k:512 * k + NCH],
                        smat[:, :, M * g:M * (g + 1)],
                        stv[:, :, NCH * k:NCH * (k + 1)],
                        start=True, stop=True, perf_mode=DR,
                        tile_position=(0, 0))
                ptv = pt.rearrange("p (u q) -> p u q", q=512)[:, :, 0:NCH]
                with nc.allow_low_precision(reason="GAP partials in bf16"):
                    if assign[3 * t + g] == 'S':
                        nc.scalar.activation(ptv, ptv, Relu,
                                             bias=cbias[:, g:g + 1],
                                             accum_out=gsum[:, g:g + 1])
                    else:
                        nc.vector.scalar_tensor_tensor(
                            out=ptv, in0=ptv,
                            scalar=cbias[:, g:g + 1], in1=zv,
                            op0=Aadd, op1=Amax,
                            accum_out=gsum[:, g:g + 1])
            return pt_g0

        def emit_scan(t, slot):
            nonlocal cprev, hprev
            gsum = gsums[t]
            gp = slot[0:96, 1960:1961]
            for g in range(3):
                nc.tensor.matmul(gp, wx[:, g, :], gsum[:, g:g + 1],
                                 start=(g == 0), stop=False,
                                 tile_position=(0, 0), skip_group_check=True)
            nc.tensor.matmul(gp, wc[:], cprev[:], start=False, stop=True,
                             tile_position=(0, 0), skip_group_check=True)
            gsums[t] = None

            # gate rows 64:96 hold 2*(a3+b3) (x2 folded into stationaries):
            # tanh(a3) = 2*sigmoid(2*a3+2*b3) - 1. All cross-gate products
            # are staged at partition base 32 (walrus requires equal operand
            # bases); single-src ACT outputs may shift partitions freely.
            # fp32 through the 2*sigma-1 cancellations; bf16 only for c/h
            sab = ga_pool.tile([2 * U, 1], FP32, tag='sab')
            nc.scalar.activation(sab[:], gp[0:2 * U, :], Sigmoid)
            s3t = ga_pool.tile([2 * U, 1], FP32, tag='s3t')
            nc.scalar.activation(s3t[U:2 * U, :], gp[2 * U:3 * U, :], Sigmoid)

            t1v = tmp.tile([2 * U, 1], FP32, tag='t1')
            nc.gpsimd.tensor_mul(t1v[U:2 * U, :], sab[0:U, :], hprev[:])
            t2v = tmp.tile([2 * U, 1], FP32, tag='t2')
            nc.gpsimd.tensor_mul(t2v[U:2 * U, :], sab[U:2 * U, :],
                                 s3t[U:2 * U, :])
            t2b = tmp.tile([2 * U, 1], FP32, tag='t2b')
            nc.vector.scalar_tensor_tensor(out=t2b[U:2 * U, :],
                                           in0=t2v[U:2 * U, :], scalar=2.0,
                                           in1=sab[U:2 * U, :],
                                           op0=Amul, op1=Asub)
            cnew = cbufs[t % 3]
            nc.vector.tensor_add(cnew[0:U, :], t1v[U:2 * U, :],
                                 t2b[U:2 * U, :])
            s4 = tmp.tile([U, 1], FP32, tag='s4')
            nc.scalar.activation(s4[:], cnew[0:U, :], Sigmoid, scale=2.0)
            w4 = tmp.tile([U, 1], FP32, tag='w4')
            nc.gpsimd.tensor_scalar(out=w4[:], in0=s4[:], scalar1=2.0,
                                    scalar2=1.0, op0=Amul, op1=Asub)
            hnew = hbufs[t % 3]
            nc.vector.tensor_mul(hnew[:], w4[:], cnew[0:U, :])
            cprev, hprev = cnew, hnew

        LAG = 3  # scan step t emitted after frame t+LAG's conv
        for t in range(T):
            pt_g0 = emit_conv(t)
            if t >= LAG:
                emit_scan(t - LAG, pt_g0)
        for t in range(T - LAG, T):
            tail = psum.tile([M, 2048], FP32, tag='pt')
            emit_scan(t, tail)

        hout = tmp.tile([U, 1], FP32, tag='hout')
        nc.scalar.activation(hout[:], hprev[:], Copy)
        nc.sync.dma_start(outh_d[:], hout[:])

        for p in (tmp, ga_pool, gs, psum, stackp, state, const):
            p.release()

    return nc


# -------------------------------------------------------------- host prep
def _prep_inputs(x, conv_w, conv_b, W1, b1, W2, b2, W3, b3):
    x = np.asarray(x, np.float32)
    conv_w = np.asarray(conv_w, np.float32)
    conv_b = np.asarray(conv_b, np.float32)

    # --- fp8 stack: [B, T, 45, 2, 1568], tap tau = c*30 + dx*10 + wr ---
    xp = np.zeros((B, T, H + 2, W + 2, C), np.float32)
    xp[:, :, 1:H + 1, 1:W + 1, :] = x
    stack = np.empty((B, T, 90, JA, W), np.float32)
    for c in range(C):
        for dx in range(3):
            for wr in range(10):
                tau = c * 30 + dx * 10 + wr
                stack[:, :, tau] = xp[:, :, wr:wr + 8 * (JA - 1) + 1:8,
                                      dx:dx + W, c]
    xin = stack.reshape(B, T, 2, KP, NCOL).transpose(0, 1, 3, 2, 4)
    xin = np.ascontiguousarray(xin).reshape(B, T, KP, 2 * NCOL)
    xin = xin.astype(ml_dtypes.float8_e4m3fn)

    # --- fp8 stationaries: [45, 2, 3*128], col j = f_loc*8 + i ---
    smat = np.zeros((90, 3, M), np.float32)
    for c in range(C):
        for dx in range(3):
            for wr in range(10):
                tau = c * 30 + dx * 10 + wr
                for i in range(8):
                    dy = wr - i
                    if 0 <= dy <= 2:
                        for g in range(3):
                            fl = np.arange(16)
                            smat[tau, g, fl * 8 + i] = conv_w[dy, dx, c,
                                                              g * 16 + fl]
    smat = smat.reshape(2, KP, 3 * M).transpose(1, 0, 2)
    smat = np.ascontiguousarray(smat).astype(ml_dtypes.float8_e4m3fn)

    cbias = np.empty((M, 3), np.float32)
    for g in range(3):
        cbias[:, g] = np.repeat(conv_b[g * 16:(g + 1) * 16], 8)

    # --- scan weights ---
    Wall = np.stack([np.asarray(Wg, np.float32) for Wg in (W1, W2, W3)], axis=1)
    Wall = Wall.reshape(F + U, 96)  # rows: feats 0..47, hidden 48..79
    wxf = Wall[0:F] / HWN           # [48, 96]
    wx = np.empty((M, 3 * 96), np.float32)
    for g in range(3):
        # partition j = f_loc*8 + i replicates the filter row across i
        wx[:, 96 * g:96 * (g + 1)] = np.repeat(wxf[g * 16:(g + 1) * 16],
                                               8, axis=0)
    gb = np.concatenate([np.asarray(b, np.float32) for b in (b1, b2, b3)])
    wc = np.concatenate([Wall[F:F + U], gb.reshape(1, 96)], axis=0)  # [33,96]
    # tanh trick: gate-3 pre-activations (cols 64:96) scaled by 2
    wx[:, 2 * 96 + 64:2 * 96 + 96] = wx[:, 2 * 96 + 64:2 * 96 + 96]
    for g in range(3):
        wx[:, 96 * g + 64:96 * g + 96] *= 2.0
    wc[:, 64:96] *= 2.0

    return (xin, smat, cbias,
            wx.astype(ml_dtypes.bfloat16), wc.astype(ml_dtypes.bfloat16))


# ------------------------------------------------------------------ kernel
def kernel(x, conv_w, conv_b, W1, b1, W2, b2, W3, b3, W4, b4):
    global LAST_RESULTS
    xin, smat, cbias, wx, wc = _prep_inputs(
        x, conv_w, conv_b, W1, b1, W2, b2, W3, b3)

    nc = _build_bass()
    in_maps = [{
        'xin': np.ascontiguousarray(xin[b]),
        'smat': smat,
        'cbias': cbias,
        'wx': wx,
        'wc': wc,
    } for b in range(B)]

    res = run_bass_kernel_spmd(nc, in_maps, core_ids=list(range(B)))
    LAST_RESULTS = res
    out = np.stack([res.results[b]['outh'][:, 0] for b in range(B)], axis=0)
    return out.astype(np.float32)
